# revision 1
# baseline (speedup 1.0000x reference)
"""AdaConv2D Trainium2 kernel: instance-norm + per-sample depthwise-separable
conv + dense 3x3 spatial conv + LeakyReLU, data-parallel over batch on 8 cores.

Per core (one batch sample):
  - z0[c] = sum_k dw[c,k] * x[c, window_k]   (raw depthwise, independent of
    instance-norm stats; first two 8-row chunks run on the otherwise-idle
    TensorE as diagonal matmuls, the rest on VectorE with f32 accumulators)
  - instance norm + pointwise fold into the matmul side:
        out = sum_ci (W * s[ci])^T @ z0[ci] + c[co],  s = pw * rsqrt(var+eps)
        c[co] = sum_ci sum_k W[co,ci,k] * beta[ci],
        beta = bias - mean * s * sum_k dw
    so the dense 3x3 512->512 conv runs on TensorE as 36 accumulating bf16
    matmuls per (4-row band, output channel group) against scaled weights,
    streaming against a 40-row z ring buffer (reflect padding = AP row
    selection, split into stride-1 runs at ring wrap / image edges).
  - stats stream as VectorE bn_stats/bn_aggr per channel group, so the scaled
    weights W'[cg] unblock group-by-group while stats still stream; TensorE
    is fully busy from ~110us.
  - LeakyReLU fused as max(0.01*v, v) on VectorE; bias c+spatial_b on ScalarE.
Weights/x are host-packed/cast to bf16 in the exact SBUF layouts used on chip.
"""
import sys
import numpy as np

sys.path.insert(0, "/opt/trn_rl_repo")

import ml_dtypes  # noqa: E402

B, C, H, W = 8, 512, 128, 128
CG = C // 128          # 4 channel groups
PIX = H * W
BAND = 4               # output rows per matmul band
NBANDS = H // BAND
ZCH = 8                # z production chunk rows
NZCH = H // ZCH
RING = 40              # z ring rows (multiple of ZCH)
WPAD = W + 2           # width-padded row length
XCH = 2048             # pass-A stats chunk (free-dim cols)
NCH = PIX // XCH       # chunks per channel group
EPS = 1e-5
SLOPE = 0.01

_CACHE = {}


def _reflect(r):
    if r < 0:
        return -r
    if r > H - 1:
        return 2 * (H - 1) - r
    return r


def _build():
    from concourse import bacc, tile, mybir

    AF = mybir.ActivationFunctionType
    ALU = mybir.AluOpType
    F32 = mybir.dt.float32
    BF16 = mybir.dt.bfloat16

    nc = bacc.Bacc(None, target_bir_lowering=False, debug=False)

    x_ext = nc.declare_dram_parameter("x", [C, PIX], BF16, isOutput=False)
    dwd_ext = nc.declare_dram_parameter("dwd", [128, CG * 9 * 128], BF16, isOutput=False)
    wt_ext = nc.declare_dram_parameter("wt", [128, CG * 9 * CG * 128], BF16, isOutput=False)
    dw_ext = nc.declare_dram_parameter("dw", [128, CG, 9], F32, isOutput=False)
    pw_ext = nc.declare_dram_parameter("pw", [128, CG], F32, isOutput=False)
    bias_ext = nc.declare_dram_parameter("bias", [128, CG], F32, isOutput=False)
    sb_ext = nc.declare_dram_parameter("sb", [128, CG], F32, isOutput=False)
    out_ext = nc.declare_dram_parameter("out", [C, PIX], F32, isOutput=True)

    with tile.TileContext(nc) as tc:
        with (
            tc.tile_pool(name="persist", bufs=1) as pp,
            tc.tile_pool(name="xa", bufs=3) as xa_pool,
            tc.tile_pool(name="scra", bufs=2) as scra_pool,
            tc.tile_pool(name="scrv", bufs=2) as scrv_pool,
            tc.tile_pool(name="xb", bufs=2) as xb_pool,
            tc.tile_pool(name="accp", bufs=2) as acc_pool,
            tc.tile_pool(name="ub", bufs=4) as ub_pool,
            tc.tile_pool(name="ob", bufs=4) as ob_pool,
            tc.tile_pool(name="psum", bufs=7, space="PSUM") as psum_pool,
            tc.tile_pool(name="cpsum", bufs=1, space="PSUM") as cpsum_pool,
        ):
            # ---------------- persistent tiles ----------------
            wt = pp.tile([128, CG, 9, CG, 128], BF16)       # lhsT tiles
            zr = pp.tile([128, CG, RING, WPAD], BF16)       # z0 ring
            sm = pp.tile([128, 160], F32)                   # packed small vectors
            dwt = pp.tile([128, CG, 9], F32)
            ws = pp.tile([128, CG, CG * 128], BF16)         # sum_k W per cgi
            btb = pp.tile([128, CG], BF16)                  # beta as bf16 (c-matmul rhs)
            bnst = pp.tile([128, CG, NCH, 4, 6], F32)       # bn_stats group triples
            dwd = pp.tile([128, CG, 9, 128], BF16)          # diag(dw) lhsT tiles

            # sm column map
            ASUM, ASQ, S2, Q2, M1, E1 = 0, 16, 28, 44, 48, 56
            MEAN, SQM, INV, SVEC, SSUM, TV, BETA, NEGV, SD, EPSC, SB, CB = (
                64, 68, 72, 76, 80, 84, 88, 92, 96, 100, 104, 108)
            NBN = 5                      # chunks on VectorE bn_stats
            NACT = NCH - NBN             # chunks on ScalarE sum/sumsq

            nc.sync.dma_start(dwt[:], dw_ext[:])
            nc.sync.dma_start(dwd[:].rearrange('p a b c -> p (a b c)'), dwd_ext[:])
            nc.sync.dma_start(sm[:, SVEC:SVEC + CG], pw_ext[:])   # stage pw in SVEC
            nc.sync.dma_start(sm[:, BETA:BETA + CG], bias_ext[:])  # stage bias in BETA
            nc.sync.dma_start(sm[:, SB:SB + CG], sb_ext[:])
            nc.gpsimd.memset(sm[:, EPSC:EPSC + 1], EPS)

            # ---------------- stats + weight prep ----------------
            def emit_dw_dma(c):
                r0 = c * ZCH
                xbt = xb_pool.tile([128, CG, ZCH + 2, WPAD], BF16, name="xbt")
                lo, hi = r0 - 1, r0 + ZCH
                dlo, dhi = max(lo, 0), min(hi, H - 1)
                for cg in range(CG):
                    src = x_ext[cg * 128:(cg + 1) * 128, :].rearrange(
                        'p (h w) -> p h w', h=H)
                    nc.sync.dma_start(xbt[:, cg, dlo - lo:dhi - lo + 1, 1:W + 1],
                                      src[:, dlo:dhi + 1, :])
                    if lo < 0:
                        nc.sync.dma_start(xbt[:, cg, 0, 1:W + 1], src[:, 1, :])
                    if hi > H - 1:
                        nc.sync.dma_start(xbt[:, cg, ZCH + 1, 1:W + 1], src[:, H - 2, :])
                nc.vector.tensor_copy(xbt[:, :, :, 0:1], xbt[:, :, :, 2:3])
                nc.vector.tensor_copy(xbt[:, :, :, W + 1:W + 2], xbt[:, :, :, W - 1:W])
                return xbt

            WTCH = 9 * CG * 128
            inv_n = 1.0 / float(PIX)
            pre_xbt = {}
            for cg in range(CG):
                # this group's weight slab first, then its stats chunks, so the
                # scaled weights W'[cg] unblock as early as possible
                nc.sync.dma_start(
                    wt[:, cg, :, :, :].rearrange('p b c d -> p (b c d)'),
                    wt_ext[:, cg * WTCH:(cg + 1) * WTCH])
                for ch in range(NCH):
                    xat = xa_pool.tile([128, XCH], BF16)
                    nc.sync.dma_start(
                        xat[:], x_ext[cg * 128:(cg + 1) * 128, ch * XCH:(ch + 1) * XCH])
                    if ch < NBN:
                        # bn_stats: one 512-col group per op (HW FMAX), gives
                        # (count, mean, count*var) triples; bn_aggr combines
                        for g in range(XCH // 512):
                            nc.vector.bn_stats(bnst[:, cg, ch, g, :],
                                               xat[:, g * 512:(g + 1) * 512])
                    else:
                        # remaining chunks on the otherwise-idle ScalarE
                        i = ch - NBN
                        scrt = scra_pool.tile([128, XCH], BF16)
                        nc.scalar.activation(
                            scrt[:], xat[:], AF.Copy,
                            accum_out=sm[:, ASUM + cg * NACT + i:ASUM + cg * NACT + i + 1])
                        scqt = scra_pool.tile([128, XCH], BF16, tag="scrt")
                        nc.scalar.activation(
                            scqt[:], xat[:], AF.Square,
                            accum_out=sm[:, ASQ + cg * NACT + i:ASQ + cg * NACT + i + 1])
                # ws[cg] = sum_k W (after the chunks: keeps chunk 0's stats ops
                # at the head of the VectorE stream instead of behind the
                # weight-slab DMA wait)
                wv = wt[:, cg, :, :, :].rearrange('p k a b -> p k (a b)')
                nc.vector.tensor_tensor(ws[:, cg, :], wv[:, 0, :], wv[:, 1, :], ALU.add)
                for k in range(2, 9):
                    nc.vector.tensor_tensor(ws[:, cg, :], ws[:, cg, :], wv[:, k, :],
                                            ALU.add)
                # merge the bn_aggr partial (n1 pixels) with the ScalarE
                # sums/sumsq partial (n2 pixels); biased var matches the ref
                n1 = float(NBN * XCH)
                nc.vector.bn_aggr(sm[:, M1 + 2 * cg:M1 + 2 * cg + 2],
                                  bnst[:, cg, 0:NBN, :, :])
                nc.vector.tensor_reduce(
                    sm[:, S2 + cg:S2 + cg + 1],
                    sm[:, ASUM + cg * NACT:ASUM + (cg + 1) * NACT],
                    mybir.AxisListType.X, ALU.add)
                nc.vector.tensor_reduce(
                    sm[:, Q2 + cg:Q2 + cg + 1],
                    sm[:, ASQ + cg * NACT:ASQ + (cg + 1) * NACT],
                    mybir.AxisListType.X, ALU.add)
                # mean = (n1*m1 + S2)/N
                nc.vector.scalar_tensor_tensor(
                    sm[:, MEAN + 2 * cg:MEAN + 2 * cg + 1],
                    sm[:, M1 + 2 * cg:M1 + 2 * cg + 1], n1,
                    sm[:, S2 + cg:S2 + cg + 1], ALU.mult, ALU.add)
                nc.vector.tensor_scalar(sm[:, MEAN + 2 * cg:MEAN + 2 * cg + 1],
                                        sm[:, MEAN + 2 * cg:MEAN + 2 * cg + 1],
                                        inv_n, None, ALU.mult)
                # e1 = m1^2 + v1 ; q = (n1*e1 + Q2)/N ; negvar = mean^2 - q
                nc.vector.scalar_tensor_tensor(
                    sm[:, E1 + cg:E1 + cg + 1], sm[:, M1 + 2 * cg:M1 + 2 * cg + 1],
                    sm[:, M1 + 2 * cg:M1 + 2 * cg + 1],
                    sm[:, M1 + 2 * cg + 1:M1 + 2 * cg + 2], ALU.mult, ALU.add)
                nc.vector.scalar_tensor_tensor(
                    sm[:, Q2 + cg:Q2 + cg + 1], sm[:, E1 + cg:E1 + cg + 1], n1,
                    sm[:, Q2 + cg:Q2 + cg + 1], ALU.mult, ALU.add)
                nc.vector.tensor_scalar(sm[:, Q2 + cg:Q2 + cg + 1],
                                        sm[:, Q2 + cg:Q2 + cg + 1],
                                        inv_n, None, ALU.mult)
                nc.vector.scalar_tensor_tensor(
                    sm[:, NEGV + cg:NEGV + cg + 1],
                    sm[:, MEAN + 2 * cg:MEAN + 2 * cg + 1],
                    sm[:, MEAN + 2 * cg:MEAN + 2 * cg + 1],
                    sm[:, Q2 + cg:Q2 + cg + 1], ALU.mult, ALU.subtract)
                nc.scalar.activation(sm[:, SD + cg:SD + cg + 1],
                                     sm[:, NEGV + cg:NEGV + cg + 1],
                                     AF.Sqrt, bias=sm[:, EPSC:EPSC + 1], scale=-1.0)
                nc.vector.reciprocal(sm[:, INV + cg:INV + cg + 1],
                                     sm[:, SD + cg:SD + cg + 1])
                # s = pw * inv (pw staged in SVEC)
                nc.vector.scalar_tensor_tensor(
                    sm[:, SVEC + cg:SVEC + cg + 1], sm[:, SVEC + cg:SVEC + cg + 1],
                    1.0, sm[:, INV + cg:INV + cg + 1], ALU.mult, ALU.mult)
                # S = sum_k dw
                nc.vector.tensor_reduce(sm[:, SSUM + cg:SSUM + cg + 1],
                                        dwt[:, cg, :], mybir.AxisListType.X, ALU.add)
                # t = mean * s * S
                nc.vector.scalar_tensor_tensor(
                    sm[:, TV + cg:TV + cg + 1], sm[:, MEAN + 2 * cg:MEAN + 2 * cg + 1],
                    1.0, sm[:, SVEC + cg:SVEC + cg + 1], ALU.mult, ALU.mult)
                nc.vector.scalar_tensor_tensor(
                    sm[:, TV + cg:TV + cg + 1], sm[:, TV + cg:TV + cg + 1],
                    1.0, sm[:, SSUM + cg:SSUM + cg + 1], ALU.mult, ALU.mult)
                # beta = bias - t   (bias staged in BETA)
                nc.vector.scalar_tensor_tensor(
                    sm[:, BETA + cg:BETA + cg + 1], sm[:, TV + cg:TV + cg + 1],
                    -1.0, sm[:, BETA + cg:BETA + cg + 1], ALU.mult, ALU.add)
                nc.vector.tensor_copy(btb[:, cg:cg + 1], sm[:, BETA + cg:BETA + cg + 1])
                # W' = W * s[ci]  (in-place; WAR on the ws reads above)
                wv = wt[:, cg, :, :, :].rearrange('p k a b -> p (k a b)')
                nc.vector.tensor_scalar(wv, wv, sm[:, SVEC + cg:SVEC + cg + 1],
                                        None, ALU.mult)
                if cg < 2:
                    # prologue x-band DMAs interleave with the stats stream so
                    # the TensorE diagonal-depthwise runs during the window
                    # instead of queueing behind all stats DMA
                    pre_xbt[cg] = emit_dw_dma(cg)

            def emit_c_mms():
                # c[co] = sum_cgi ws[cgi]^T @ beta[cgi]; then CB = c + spatial_b.
                # Reserved psum bank: band chains must not wait on this slot
                # (its evict depends on stats; a shared slot would deadlock the
                # staged chains whose evicts read CB).
                cpt = cpsum_pool.tile([128, 512], F32)
                for cgo in range(CG):
                    for cgi in range(CG):
                        nc.tensor.matmul(cpt[:, cgo:cgo + 1],
                                         ws[:, cgi, cgo * 128:(cgo + 1) * 128],
                                         btb[:, cgi:cgi + 1],
                                         start=(cgi == 0), stop=(cgi == CG - 1),
                                         skip_group_check=(cgo != 0 or cgi != 0))
                for cgo in range(CG):
                    nc.scalar.activation(sm[:, CB + cgo:CB + cgo + 1],
                                         cpt[:, cgo:cgo + 1],
                                         AF.Identity,
                                         bias=sm[:, SB + cgo:SB + cgo + 1],
                                         scale=1.0)

            # ---------------- z0 production (chunks of 8 rows) ----------------
            def emit_dw_chunk(c, on_pe=False, xbt=None):
                r0 = c * ZCH
                if xbt is None:
                    xbt = emit_dw_dma(c)
                s0 = r0 % RING
                for cg in range(CG):
                    if on_pe:
                        # depthwise on the (otherwise idle) TensorE as diagonal
                        # matmuls — removes ~37us/chunk from VectorE's critical
                        # path during the stats prologue; ScalarE evicts.
                        for half in range(2):
                            pt = psum_pool.tile([128, BAND * W], F32, tag="pt",
                                                name=f"zp{c}_{cg}_{half}")
                            for k in range(9):
                                kh, kw = k // 3, k % 3
                                rhs = xbt[:, cg, kh + 4 * half:kh + 4 * half + 4,
                                          kw:kw + W]
                                nc.tensor.matmul(pt[:], dwd[:, cg, k, :], rhs,
                                                 start=(k == 0), stop=(k == 8))
                            nc.scalar.activation(
                                zr[:, cg, s0 + 4 * half:s0 + 4 * half + 4, 1:W + 1],
                                pt[:].rearrange('p (a b) -> p a b', a=BAND),
                                AF.Copy)
                    else:
                        acct = acc_pool.tile([128, ZCH, W], F32)
                        for kh in range(3):
                            for kw in range(3):
                                k = kh * 3 + kw
                                xwin = xbt[:, cg, kh:kh + ZCH, kw:kw + W]
                                gs = dwt[:, cg, k:k + 1]
                                if k == 0:
                                    nc.vector.tensor_scalar(acct[:], xwin, gs, None,
                                                            ALU.mult)
                                elif k < 8:
                                    nc.vector.scalar_tensor_tensor(
                                        acct[:], xwin, gs, acct[:], ALU.mult, ALU.add)
                                else:
                                    nc.vector.scalar_tensor_tensor(
                                        zr[:, cg, s0:s0 + ZCH, 1:W + 1], xwin, gs,
                                        acct[:], ALU.mult, ALU.add)
                    nc.vector.tensor_copy(zr[:, cg, s0:s0 + ZCH, 0:1],
                                          zr[:, cg, s0:s0 + ZCH, 2:3])
                    nc.vector.tensor_copy(zr[:, cg, s0:s0 + ZCH, W + 1:W + 2],
                                          zr[:, cg, s0:s0 + ZCH, W - 1:W])

            def slot_runs(r0, kh):
                slots = [_reflect(r0 - 1 + kh + i) % RING for i in range(BAND)]
                runs = []
                i = 0
                while i < BAND:
                    j = i
                    while j + 1 < BAND and slots[j + 1] == slots[j] + 1:
                        j += 1
                    runs.append((slots[i], i, j - i + 1))
                    i = j + 1
                return runs

            def chain_plans(b):
                r0 = b * BAND
                s0 = r0 % RING
                ordered = [(1, 1)] + [(kh, kw) for kh in range(3)
                                      for kw in range(3) if (kh, kw) != (1, 1)]
                plans = []
                for cgi in range(CG):
                    for kh, kw in ordered:
                        runs = ([(s0, 0, BAND)] if kh == 1 else slot_runs(r0, kh))
                        for (sl, off, ln) in runs:
                            plans.append((kh, kw, cgi, sl, off, ln))
                return plans

            def emit_chain_mms(pt, b, cgo, plans, lo, hi):
                total = len(plans)
                for idx in range(lo, hi):
                    kh, kw, cgi, sl, off, ln = plans[idx]
                    rhs = zr[:, cgi, sl:sl + ln, kw:kw + W]
                    lhsT = wt[:, cgi, kh * 3 + kw, cgo, :]
                    nc.tensor.matmul(pt[:, off * W:(off + ln) * W], lhsT, rhs,
                                     start=(idx == 0), stop=(idx == total - 1),
                                     skip_group_check=(idx != 0))

            def emit_evict(pt, b, cgo):
                r0 = b * BAND
                ut = ub_pool.tile([128, BAND * W], F32)
                nc.scalar.activation(ut[:], pt[:], AF.Identity,
                                     bias=sm[:, CB + cgo:CB + cgo + 1], scale=1.0)
                ot = ob_pool.tile([128, BAND * W], F32)
                nc.vector.scalar_tensor_tensor(ot[:], ut[:], SLOPE, ut[:],
                                               ALU.mult, ALU.max)
                nc.sync.dma_start(
                    out_ext[cgo * 128:(cgo + 1) * 128,
                            r0 * W:(r0 + BAND) * W], ot[:])

            def emit_mm_band(b):
                for cgo in range(CG):
                    pt = psum_pool.tile([128, BAND * W], F32, tag="pt")
                    plans = chain_plans(b)
                    emit_chain_mms(pt, b, cgo, plans, 0, len(plans))
                    emit_evict(pt, b, cgo)

            emit_dw_chunk(0, on_pe=True, xbt=pre_xbt[0])
            emit_dw_chunk(1, on_pe=True, xbt=pre_xbt[1])
            emitted_chunk = 1
            for b in range(NBANDS):
                need = min((b + 1) // 2 + 1, NZCH - 1)
                while emitted_chunk < need:
                    emitted_chunk += 1
                    emit_dw_chunk(emitted_chunk)
                if b == 0:
                    # band 0 special-cased: chains first, then the stats-gated
                    # c-matmuls (so they don't head-of-line-stall the TensorE
                    # stream at chain start), then the evicts that read CB.
                    # Emission order must keep CB's writers before its readers.
                    pts0 = []
                    pl0 = chain_plans(0)
                    for cgo in range(CG):
                        pt = psum_pool.tile([128, BAND * W], F32, tag="pt",
                                            name=f"b0pt{cgo}")
                        emit_chain_mms(pt, 0, cgo, pl0, 0, len(pl0))
                        pts0.append(pt)
                    emit_c_mms()
                    for cgo in range(CG):
                        emit_evict(pts0[cgo], 0, cgo)
                else:
                    emit_mm_band(b)

    nc.compile()
    return nc


def _get_nc():
    if "nc" not in _CACHE:
        _CACHE["nc"] = _build()
    return _CACHE["nc"]


def _pack_inputs(x, dw_kernels, pw_kernels, biases, spatial_w, spatial_b):
    """Host-side layout packing (no reference math, just reorder/cast)."""
    w = np.asarray(spatial_w, dtype=np.float32).reshape(CG, 128, CG, 128, 9)
    # dims: (cgo, co, cgi, ci, k) -> (ci, cgi, k, cgo, co)
    wt = np.ascontiguousarray(w.transpose(3, 2, 4, 0, 1)).astype(ml_dtypes.bfloat16)
    wt = wt.reshape(128, CG * 9 * CG * 128)

    in_maps = []
    for b in range(B):
        xb = np.ascontiguousarray(
            np.asarray(x[b], dtype=np.float32).reshape(C, PIX)).astype(
                ml_dtypes.bfloat16)
        dwb = np.asarray(dw_kernels[b], dtype=np.float32).reshape(CG, 128, 9)
        dwb = np.ascontiguousarray(dwb.transpose(1, 0, 2))            # [128, CG, 9]
        # diag(dw) lhsT tiles for the TensorE depthwise prologue
        dwd = np.zeros((128, CG, 9, 128), dtype=np.float32)
        ii = np.arange(128)
        dwd[ii, :, :, ii] = dwb
        dwd = dwd.astype(ml_dtypes.bfloat16).reshape(128, CG * 9 * 128)
        pwb = np.asarray(pw_kernels[b], dtype=np.float32).reshape(CG, 128).T
        bb = np.asarray(biases[b], dtype=np.float32).reshape(CG, 128).T
        sbb = np.asarray(spatial_b, dtype=np.float32).reshape(CG, 128).T
        in_maps.append({
            "x": xb,
            "wt": wt,
            "dwd": np.ascontiguousarray(dwd),
            "dw": np.ascontiguousarray(dwb),
            "pw": np.ascontiguousarray(pwb),
            "bias": np.ascontiguousarray(bb),
            "sb": np.ascontiguousarray(sbb),
        })
    return in_maps


def _run(inputs, trace=False):
    from concourse.bass_utils import run_bass_kernel_spmd
    if trace:
        _install_trace_hook()
    nc = _get_nc()
    in_maps = _pack_inputs(**inputs)
    res = run_bass_kernel_spmd(nc, in_maps, core_ids=list(range(B)), trace=trace)
    out = np.stack([res.results[i]["out"].reshape(C, H, W) for i in range(B)])
    return out, res


def _install_trace_hook():
    import types
    try:
        import antenv.axon_hooks  # noqa
    except ImportError:
        from trn_agent_boot.trn_boot import _ntff_profile_via_ctypes
        hook = _ntff_profile_via_ctypes('/opt/axon/libaxon_pjrt.so')
        mod = types.ModuleType('antenv.axon_hooks')
        mod.get_axon_ntff_profile_hook = lambda: hook
        mod.set_axon_ntff_profile_hook = lambda h: None
        sys.modules['antenv.axon_hooks'] = mod


def kernel(**inputs):
    out, _ = _run(inputs, trace=False)
    return out



# revision 12
# speedup vs baseline: 1.0420x; 1.0420x over previous
"""AdaConv2D Trainium2 kernel, v2: instance-norm + per-sample depthwise-separable
conv + dense 3x3 spatial conv + LeakyReLU, data-parallel over batch on 8 cores.

Key idea vs v1 (direct conv, ~1104us): the dense 3x3 conv runs as 1D Winograd
F(2,3) along W, cutting TensorE matmul rows from 9/px to 6/px (~2/3).

Per core (one batch sample):
  - x is host-packed even/odd-split per row: [O(65) | E(65)] (130 cols), with
    reflect col pads baked in. All on-chip DVE taps/combos read/write packed
    slices so the 2x/4x DVE perf modes engage.
  - z0 = raw depthwise (stats-independent): chunks 0..3 on TensorE as diagonal
    matmuls during the stats prologue; chunks 4..15 on VectorE as
    tensor_scalar (4x) + tensor_tensor (2x) chains per parity.
  - Winograd input transform V0..V3 = {zO-zO', zE+zO', zO'-zE, zE-zE'} on
    GpSimd (tensor_tensor, SBUF-only) into a V ring buffer.
  - dense conv: per 8-row band x cgo: 4 psum tiles M0..M3, each an
    accumulation chain of 12 matmuls (3 kh x 4 cgi) of free 512 against
    host-transformed weights W~ = G(2,3) @ W scaled on-chip by
    s = pw*rsqrt(var+eps) (instance norm + pointwise folded into lhsT).
  - output: out_even = M0+M1+M2, out_odd = M1-M2-M3 on VectorE (interleaved
    strided write), then LeakyReLU with bias c[co] fused on ScalarE (AF.Lrelu),
    bf16 DMA out (host casts f32).
  - c[co] = sum_ci (sum_k W)^T @ beta, beta = bias - mean*s*sum(dw), computed
    by TensorE right after the prologue (same stats gate as band 0).
"""
import sys
import numpy as np

sys.path.insert(0, "/opt/trn_rl_repo")

import ml_dtypes  # noqa: E402

B, C, H, W = 8, 512, 128, 128
CG = C // 128          # 4 channel groups
PIX = H * W
NT = 64                # winograd tiles (output col pairs) per row
BAND = 8               # output rows per matmul band
NBANDS = H // BAND
ZCH = 8                # z production chunk rows
NZCH = H // ZCH
PE_CHUNKS = 3          # leading chunks on TensorE (diag depthwise); must not
                       # exceed RINGV//ZCH (ring would alias before band 0)
RINGV = 24             # V ring rows (multiple of ZCH)
WROW = 130             # packed row: [O(65) | E(65)]
HO, HE = 0, 65         # offsets of odd / even segments in a packed row
XROWS = 16             # stats chunk rows
NCH = H // XROWS       # 8 stats chunks per channel group
NBN = 3                # stats chunks on VectorE bn_stats
NACT = NCH - NBN       # stats chunks on ScalarE accum
EPS = 1e-5
SLOPE = 0.01

_CACHE = {}


def _reflect(r):
    if r < 0:
        return -r
    if r > H - 1:
        return 2 * (H - 1) - r
    return r


def _build():
    from concourse import bacc, tile, mybir

    AF = mybir.ActivationFunctionType
    ALU = mybir.AluOpType
    F32 = mybir.dt.float32
    BF16 = mybir.dt.bfloat16

    nc = bacc.Bacc(None, target_bir_lowering=False, debug=False)

    x_ext = nc.declare_dram_parameter("x", [C, H * WROW], BF16, isOutput=False)
    wt_ext = nc.declare_dram_parameter("wt", [128, CG * 3 * 4 * C], BF16, isOutput=False)
    ws_ext = nc.declare_dram_parameter("ws", [128, CG * C], BF16, isOutput=False)
    dwd_ext = nc.declare_dram_parameter("dwd", [128, CG * 9 * 128], BF16, isOutput=False)
    dw_ext = nc.declare_dram_parameter("dw", [128, CG, 9], F32, isOutput=False)
    pw_ext = nc.declare_dram_parameter("pw", [128, CG], F32, isOutput=False)
    bias_ext = nc.declare_dram_parameter("bias", [128, CG], F32, isOutput=False)
    sb_ext = nc.declare_dram_parameter("sb", [128, CG], F32, isOutput=False)
    out_ext = nc.declare_dram_parameter("out", [C, PIX], BF16, isOutput=True)

    with tile.TileContext(nc) as tc:
        with (
            tc.tile_pool(name="persist", bufs=1) as pp,
            tc.tile_pool(name="xa", bufs=3) as xa_pool,
            tc.tile_pool(name="scra", bufs=2) as scra_pool,
            tc.tile_pool(name="xw", bufs=2) as xw_pool,
            tc.tile_pool(name="zb", bufs=2) as zb_pool,
            tc.tile_pool(name="ztmp", bufs=3) as ztmp_pool,
            tc.tile_pool(name="et", bufs=8) as et_pool,
            tc.tile_pool(name="ub", bufs=3) as ub_pool,
            tc.tile_pool(name="ob", bufs=3) as ob_pool,
            tc.tile_pool(name="psum", bufs=7, space="PSUM") as psum_pool,
            tc.tile_pool(name="cpsum", bufs=1, space="PSUM") as cpsum_pool,
        ):
            # ---------------- persistent tiles ----------------
            wt = pp.tile([128, CG, 3, 4, C], BF16)          # W~ lhsT [ci|cgi,kh,p,cgo*co]
            ws = pp.tile([128, CG, C], BF16)                # sum_k W per cgi (host)
            vr = pp.tile([128, CG, 4, RINGV, NT], BF16)     # V ring
            sm = pp.tile([128, 160], F32)                   # packed small vectors
            dwt = pp.tile([128, CG, 9], F32)
            btb = pp.tile([128, CG], BF16)                  # beta as bf16 (c-matmul rhs)
            bnst = pp.tile([128, CG, NBN, XROWS, 6], F32)   # bn_stats group triples
            dwd = pp.tile([128, CG, 9, 128], BF16)          # diag(dw) lhsT tiles

            # sm column map
            ASUM, ASQ = 0, 24
            S2, Q2, M1, E1 = 48, 52, 56, 64
            MEAN, SQM, INV, SVEC, SSUM, TV, BETA, NEGV, SD, EPSC, SB, CB = (
                68, 72, 76, 80, 84, 88, 92, 96, 100, 104, 108, 112)

            nc.sync.dma_start(dwt[:], dw_ext[:])
            nc.sync.dma_start(dwd[:].rearrange('p a b c -> p (a b c)'), dwd_ext[:])
            nc.sync.dma_start(ws[:].rearrange('p a b -> p (a b)'), ws_ext[:])
            nc.sync.dma_start(sm[:, SVEC:SVEC + CG], pw_ext[:])   # stage pw in SVEC
            nc.sync.dma_start(sm[:, BETA:BETA + CG], bias_ext[:])  # stage bias in BETA
            nc.sync.dma_start(sm[:, SB:SB + CG], sb_ext[:])
            nc.gpsimd.memset(sm[:, EPSC:EPSC + 1], EPS)

            xv = x_ext[:, :].rearrange('p (h w) -> p h w', h=H)

            def emit_xwin_dma(c):
                r0 = c * ZCH
                xwt = xw_pool.tile([128, CG, ZCH + 2, WROW], BF16, tag="xwt", name=f"xw{c}")
                lo, hi = r0 - 1, r0 + ZCH
                dlo, dhi = max(lo, 0), min(hi, H - 1)
                for cg in range(CG):
                    src = xv[cg * 128:(cg + 1) * 128, :, :]
                    nc.sync.dma_start(xwt[:, cg, dlo - lo:dhi - lo + 1, :],
                                      src[:, dlo:dhi + 1, :])
                    if lo < 0:
                        nc.sync.dma_start(xwt[:, cg, 0, :], src[:, 1, :])
                    if hi > H - 1:
                        nc.sync.dma_start(xwt[:, cg, ZCH + 1, :], src[:, H - 2, :])
                return xwt

            # ---------------- stats + weight prep ----------------
            WTCH = 3 * 4 * C
            inv_n = 1.0 / float(PIX)
            n1 = float(NBN * XROWS * W)
            pre_xw = {}
            for cg in range(CG):
                # this group's weight slab first, then its stats chunks
                nc.sync.dma_start(
                    wt[:, cg, :, :, :].rearrange('p a b c -> p (a b c)'),
                    wt_ext[:, cg * WTCH:(cg + 1) * WTCH])
                for ch in range(NCH):
                    xat = xa_pool.tile([128, XROWS * WROW], BF16, tag="xat")
                    nc.sync.dma_start(
                        xat[:], x_ext[cg * 128:(cg + 1) * 128,
                                      ch * XROWS * WROW:(ch + 1) * XROWS * WROW])
                    # pad-skipping view: cols 1..128 of each 130-wide row are
                    # exactly the 128 real pixels (O[1:65] ++ E[0:64])
                    xvv = xat[:].rearrange('p (r w) -> p r w', w=WROW)
                    if ch < NBN:
                        # verifier requires exactly one 6-elem group per op
                        for g in range(XROWS):
                            nc.vector.bn_stats(
                                bnst[:, cg, ch, g, :],
                                xvv[:, g, 1:1 + W])
                    else:
                        i = ch - NBN
                        scrt = scra_pool.tile([128, XROWS * W], BF16, name="scrt")
                        sv = scrt[:].rearrange('p (r w) -> p r w', w=W)
                        nc.scalar.activation(
                            sv, xvv[:, :, 1:1 + W], AF.Copy,
                            accum_out=sm[:, ASUM + cg * NACT + i:ASUM + cg * NACT + i + 1])
                        scqt = scra_pool.tile([128, XROWS * W], BF16, tag="scrt")
                        sq = scqt[:].rearrange('p (r w) -> p r w', w=W)
                        nc.scalar.activation(
                            sq, xvv[:, :, 1:1 + W], AF.Square,
                            accum_out=sm[:, ASQ + cg * NACT + i:ASQ + cg * NACT + i + 1])
                nc.vector.bn_aggr(sm[:, M1 + 2 * cg:M1 + 2 * cg + 2],
                                  bnst[:, cg, 0:NBN, :, :])
                nc.vector.tensor_reduce(
                    sm[:, S2 + cg:S2 + cg + 1],
                    sm[:, ASUM + cg * NACT:ASUM + (cg + 1) * NACT],
                    mybir.AxisListType.X, ALU.add)
                nc.vector.tensor_reduce(
                    sm[:, Q2 + cg:Q2 + cg + 1],
                    sm[:, ASQ + cg * NACT:ASQ + (cg + 1) * NACT],
                    mybir.AxisListType.X, ALU.add)
                # mean = (n1*m1 + S2)/N
                nc.vector.scalar_tensor_tensor(
                    sm[:, MEAN + cg:MEAN + cg + 1],
                    sm[:, M1 + 2 * cg:M1 + 2 * cg + 1], n1,
                    sm[:, S2 + cg:S2 + cg + 1], ALU.mult, ALU.add)
                nc.vector.tensor_scalar(sm[:, MEAN + cg:MEAN + cg + 1],
                                        sm[:, MEAN + cg:MEAN + cg + 1],
                                        inv_n, None, ALU.mult)
                # e1 = m1^2 + v1 ; q = (n1*e1 + Q2)/N ; negvar = mean^2 - q
                nc.vector.scalar_tensor_tensor(
                    sm[:, E1 + cg:E1 + cg + 1], sm[:, M1 + 2 * cg:M1 + 2 * cg + 1],
                    sm[:, M1 + 2 * cg:M1 + 2 * cg + 1],
                    sm[:, M1 + 2 * cg + 1:M1 + 2 * cg + 2], ALU.mult, ALU.add)
                nc.vector.scalar_tensor_tensor(
                    sm[:, Q2 + cg:Q2 + cg + 1], sm[:, E1 + cg:E1 + cg + 1], n1,
                    sm[:, Q2 + cg:Q2 + cg + 1], ALU.mult, ALU.add)
                nc.vector.tensor_scalar(sm[:, Q2 + cg:Q2 + cg + 1],
                                        sm[:, Q2 + cg:Q2 + cg + 1],
                                        inv_n, None, ALU.mult)
                nc.vector.scalar_tensor_tensor(
                    sm[:, NEGV + cg:NEGV + cg + 1],
                    sm[:, MEAN + cg:MEAN + cg + 1],
                    sm[:, MEAN + cg:MEAN + cg + 1],
                    sm[:, Q2 + cg:Q2 + cg + 1], ALU.mult, ALU.subtract)
                nc.scalar.activation(sm[:, SD + cg:SD + cg + 1],
                                     sm[:, NEGV + cg:NEGV + cg + 1],
                                     AF.Sqrt, bias=sm[:, EPSC:EPSC + 1], scale=-1.0)
                nc.vector.reciprocal(sm[:, INV + cg:INV + cg + 1],
                                     sm[:, SD + cg:SD + cg + 1])
                # s = pw * inv (pw staged in SVEC)
                nc.vector.scalar_tensor_tensor(
                    sm[:, SVEC + cg:SVEC + cg + 1], sm[:, SVEC + cg:SVEC + cg + 1],
                    1.0, sm[:, INV + cg:INV + cg + 1], ALU.mult, ALU.mult)
                # S = sum_k dw
                nc.vector.tensor_reduce(sm[:, SSUM + cg:SSUM + cg + 1],
                                        dwt[:, cg, :], mybir.AxisListType.X, ALU.add)
                # t = mean * s * S
                nc.vector.scalar_tensor_tensor(
                    sm[:, TV + cg:TV + cg + 1], sm[:, MEAN + cg:MEAN + cg + 1],
                    1.0, sm[:, SVEC + cg:SVEC + cg + 1], ALU.mult, ALU.mult)
                nc.vector.scalar_tensor_tensor(
                    sm[:, TV + cg:TV + cg + 1], sm[:, TV + cg:TV + cg + 1],
                    1.0, sm[:, SSUM + cg:SSUM + cg + 1], ALU.mult, ALU.mult)
                # beta = bias - t   (bias staged in BETA)
                nc.vector.scalar_tensor_tensor(
                    sm[:, BETA + cg:BETA + cg + 1], sm[:, TV + cg:TV + cg + 1],
                    -1.0, sm[:, BETA + cg:BETA + cg + 1], ALU.mult, ALU.add)
                nc.vector.tensor_copy(btb[:, cg:cg + 1], sm[:, BETA + cg:BETA + cg + 1])
                # W~' = W~ * s[ci]  (in-place, 4x-mode tensor_scalar)
                wv = wt[:, cg, :, :, :].rearrange('p a b c -> p (a b c)')
                nc.vector.tensor_scalar(wv, wv, sm[:, SVEC + cg:SVEC + cg + 1],
                                        None, ALU.mult)
                if cg < PE_CHUNKS:
                    pre_xw[cg] = emit_xwin_dma(cg)

            # ---------------- z chunk -> V ring ----------------
            # tap table: per output parity, the 9 (kh, src-offset) terms.
            # z_e[i] (i=0..63): k(kh,0)*O[i] + k(kh,1)*E[i] + k(kh,2)*O[i+1]
            # z_o[i] (i=1..64): k(kh,0)*E[i-1] + k(kh,1)*O[i] + k(kh,2)*E[i]
            TAPS_E = [(kh, kw, [HO + 0, HE + 0, HO + 1][kw]) for kh in range(3)
                      for kw in range(3)]
            TAPS_O = [(kh, kw, [HE + 0, HO + 1, HE + 1][kw]) for kh in range(3)
                      for kw in range(3)]

            def emit_combos(c, zt):
                # fixups: zO[0] <- zO[1] (col -1 = reflect col 1),
                #         zE[64] <- zE[63] (col 128 = reflect col 126)
                s0 = (c * ZCH) % RINGV
                for cg in range(CG):
                    nc.gpsimd.tensor_copy(zt[:, cg, :, HO:HO + 1],
                                          zt[:, cg, :, HO + 1:HO + 2])
                    nc.gpsimd.tensor_copy(zt[:, cg, :, HE + 64:HE + 65],
                                          zt[:, cg, :, HE + 63:HE + 64])
                    zo0 = zt[:, cg, :, HO + 0:HO + NT]
                    zo1 = zt[:, cg, :, HO + 1:HO + 1 + NT]
                    ze0 = zt[:, cg, :, HE + 0:HE + NT]
                    ze1 = zt[:, cg, :, HE + 1:HE + 1 + NT]
                    nc.gpsimd.tensor_tensor(vr[:, cg, 0, s0:s0 + ZCH, :],
                                            zo0, zo1, ALU.subtract)
                    nc.gpsimd.tensor_tensor(vr[:, cg, 1, s0:s0 + ZCH, :],
                                            ze0, zo1, ALU.add)
                    nc.gpsimd.tensor_tensor(vr[:, cg, 2, s0:s0 + ZCH, :],
                                            zo1, ze0, ALU.subtract)
                    nc.gpsimd.tensor_tensor(vr[:, cg, 3, s0:s0 + ZCH, :],
                                            ze0, ze1, ALU.subtract)

            def emit_dw_pe(c, xwt):
                # depthwise on TensorE as diagonal matmuls (prologue filler)
                zt = zb_pool.tile([128, CG, ZCH, WROW], BF16, tag="zt", name=f"ztp{c}")
                for cg in range(CG):
                    for par, taps, dsto in ((0, TAPS_E, HE + 0), (1, TAPS_O, HO + 1)):
                        pt = psum_pool.tile([128, ZCH * NT], F32, tag="pt",
                                            name=f"zp{c}_{cg}_{par}")
                        for k, (kh, kw, srco) in enumerate(taps):
                            rhs = xwt[:, cg, kh:kh + ZCH, srco:srco + NT]
                            nc.tensor.matmul(pt[:], dwd[:, cg, kh * 3 + kw, :], rhs,
                                             start=(k == 0), stop=(k == 8))
                        nc.scalar.activation(
                            zt[:, cg, :, dsto:dsto + NT],
                            pt[:].rearrange('p (a b) -> p a b', a=ZCH),
                            AF.Copy)
                emit_combos(c, zt)

            def emit_dw_dve(c, xwt=None):
                if xwt is None:
                    xwt = emit_xwin_dma(c)
                zt = zb_pool.tile([128, CG, ZCH, WROW], BF16, tag="zt", name=f"ztv{c}")
                for cg in range(CG):
                    for par, taps, dsto in ((0, TAPS_E, HE + 0), (1, TAPS_O, HO + 1)):
                        acc = zt[:, cg, :, dsto:dsto + NT]
                        for k, (kh, kw, srco) in enumerate(taps):
                            src = xwt[:, cg, kh:kh + ZCH, srco:srco + NT]
                            gs = dwt[:, cg, kh * 3 + kw:kh * 3 + kw + 1]
                            if k == 0:
                                nc.vector.tensor_scalar(acc, src, gs, None, ALU.mult)
                            else:
                                tmp = ztmp_pool.tile([128, ZCH, NT], BF16, tag="zt")
                                nc.vector.tensor_scalar(tmp[:], src, gs, None, ALU.mult)
                                nc.vector.tensor_tensor(acc, acc, tmp[:], ALU.add)
                emit_combos(c, zt)

            def emit_c_mms():
                # c[co] = sum_cgi ws[cgi]^T @ beta[cgi]; then CB = c + spatial_b.
                cpt = cpsum_pool.tile([128, C], F32)
                for cgo in range(CG):
                    for cgi in range(CG):
                        nc.tensor.matmul(cpt[:, cgo:cgo + 1],
                                         ws[:, cgi, cgo * 128:(cgo + 1) * 128],
                                         btb[:, cgi:cgi + 1],
                                         start=(cgi == 0), stop=(cgi == CG - 1),
                                         skip_group_check=(cgo != 0 or cgi != 0))
                for cgo in range(CG):
                    nc.scalar.activation(sm[:, CB + cgo:CB + cgo + 1],
                                         cpt[:, cgo:cgo + 1],
                                         AF.Identity,
                                         bias=sm[:, SB + cgo:SB + cgo + 1],
                                         scale=1.0)

            # ---------------- bands ----------------
            def slot_runs(r0, kh):
                rows = [_reflect(r0 - 1 + kh + i) % RINGV for i in range(BAND)]
                runs = []
                i = 0
                while i < BAND:
                    j = i
                    while j + 1 < BAND and rows[j + 1] == rows[j] + 1:
                        j += 1
                    runs.append((rows[i], i, j - i + 1))
                    i = j + 1
                return runs

            def emit_mm_band(b):
                r0 = b * BAND
                for cgo in range(CG):
                    pts = []
                    for p in range(4):
                        pt = psum_pool.tile([128, BAND * NT], F32, tag="pt",
                                            name=f"m{b}_{cgo}_{p}")
                        plans = []
                        for kh in range(3):
                            for cgi in range(CG):
                                for (sl, off, ln) in slot_runs(r0, kh):
                                    plans.append((kh, cgi, sl, off, ln))
                        total = len(plans)
                        for idx, (kh, cgi, sl, off, ln) in enumerate(plans):
                            rhs = vr[:, cgi, p, sl:sl + ln, :]
                            lhsT = wt[:, cgi, kh, p, cgo * 128:(cgo + 1) * 128]
                            nc.tensor.matmul(pt[:, off * NT:(off + ln) * NT],
                                             lhsT, rhs,
                                             start=(idx == 0), stop=(idx == total - 1),
                                             skip_group_check=(idx != 0))
                        pts.append(pt)
                    # evict: out_e = M0+M1+M2, out_o = M1-M2-M3, interleave.
                    # BIR forbids 2 PSUM operands per op, so ScalarE stages
                    # M1, M2 into SBUF first.
                    ut = ub_pool.tile([128, BAND, W], BF16, tag="ut")
                    uv = ut[:].rearrange('p r (c t) -> p r c t', t=2)
                    s1 = et_pool.tile([128, BAND * NT], F32, tag="et")
                    nc.scalar.activation(s1[:], pts[1][:], AF.Copy)
                    s2 = et_pool.tile([128, BAND * NT], F32, tag="et")
                    nc.scalar.activation(s2[:], pts[2][:], AF.Copy)
                    e1 = et_pool.tile([128, BAND * NT], F32, tag="et")
                    nc.vector.tensor_tensor(e1[:], pts[0][:], s1[:], ALU.add)
                    nc.vector.tensor_tensor(
                        uv[:, :, :, 0:1].rearrange('p r c t -> p (r c t)'),
                        e1[:], s2[:], ALU.add)
                    e2 = et_pool.tile([128, BAND * NT], F32, tag="et")
                    nc.vector.tensor_tensor(e2[:], s1[:], s2[:], ALU.subtract)
                    nc.vector.tensor_tensor(
                        uv[:, :, :, 1:2].rearrange('p r c t -> p (r c t)'),
                        e2[:], pts[3][:], ALU.subtract)
                    ot = ob_pool.tile([128, BAND * W], BF16, tag="ot")
                    nc.scalar.activation(ot[:], ut[:].rearrange('p r c -> p (r c)'),
                                         AF.Lrelu,
                                         bias=sm[:, CB + cgo:CB + cgo + 1],
                                         scale=1.0, alpha=SLOPE)
                    nc.sync.dma_start(
                        out_ext[cgo * 128:(cgo + 1) * 128,
                                r0 * W:(r0 + BAND) * W], ot[:])

            for c in range(PE_CHUNKS):
                emit_dw_pe(c, pre_xw[c])
            emit_c_mms()
            emitted = PE_CHUNKS - 1
            for b in range(NBANDS):
                need = min(b + 1, NZCH - 1)
                while emitted < need:
                    emitted += 1
                    emit_dw_dve(emitted)
                emit_mm_band(b)

    nc.compile()
    return nc


def _get_nc():
    if "nc" not in _CACHE:
        _CACHE["nc"] = _build()
    return _CACHE["nc"]


def _pack_inputs(x, dw_kernels, pw_kernels, biases, spatial_w, spatial_b):
    """Host-side packing: layout/cast + standard conv weight transforms."""
    bf = ml_dtypes.bfloat16
    w = np.asarray(spatial_w, dtype=np.float32).reshape(CG, 128, CG, 128, 3, 3)
    # Winograd F(2,3) weight transform along kw: p = 0..3
    W0 = w[..., 0]
    W1 = w[..., 1]
    W2 = w[..., 2]
    Wt = np.stack([W0, (W0 + W1 + W2) * 0.5, (W0 - W1 + W2) * 0.5, W2],
                  axis=-1)                     # [cgo,co,cgi,ci,kh,p]
    # dims -> (ci, cgi, kh, p, cgo, co)
    wt = np.ascontiguousarray(Wt.transpose(3, 2, 4, 5, 0, 1)).astype(bf)
    wt = wt.reshape(128, CG * 3 * 4 * C)
    wsum = w.sum(axis=(4, 5))                  # [cgo,co,cgi,ci]
    ws = np.ascontiguousarray(wsum.transpose(3, 2, 0, 1)).astype(bf)
    ws = ws.reshape(128, CG * C)

    in_maps = []
    for b in range(B):
        xb = np.asarray(x[b], dtype=np.float32)        # [C,H,W]
        xs = np.zeros((C, H, WROW), dtype=np.float32)
        xs[:, :, HO + 1:HO + 65] = xb[:, :, 1::2]      # O[i]=x[2i-1], i=1..64
        xs[:, :, HO] = xb[:, :, 1]                     # O[0]: reflect col -1
        xs[:, :, HE:HE + 64] = xb[:, :, 0::2]          # E[i]=x[2i], i=0..63
        xs[:, :, HE + 64] = xb[:, :, 126]              # E[64]: reflect col 128
        xsb = np.ascontiguousarray(xs.reshape(C, H * WROW)).astype(bf)
        dwb = np.asarray(dw_kernels[b], dtype=np.float32).reshape(CG, 128, 9)
        dwb = np.ascontiguousarray(dwb.transpose(1, 0, 2))            # [128, CG, 9]
        dwd = np.zeros((128, CG, 9, 128), dtype=np.float32)
        ii = np.arange(128)
        dwd[ii, :, :, ii] = dwb
        dwd = dwd.astype(bf).reshape(128, CG * 9 * 128)
        pwb = np.asarray(pw_kernels[b], dtype=np.float32).reshape(CG, 128).T
        bb = np.asarray(biases[b], dtype=np.float32).reshape(CG, 128).T
        sbb = np.asarray(spatial_b, dtype=np.float32).reshape(CG, 128).T
        in_maps.append({
            "x": xsb,
            "wt": wt,
            "ws": ws,
            "dwd": np.ascontiguousarray(dwd),
            "dw": np.ascontiguousarray(dwb),
            "pw": np.ascontiguousarray(pwb),
            "bias": np.ascontiguousarray(bb),
            "sb": np.ascontiguousarray(sbb),
        })
    return in_maps


def _run(inputs, trace=False):
    from concourse.bass_utils import run_bass_kernel_spmd
    if trace:
        _install_trace_hook()
    nc = _get_nc()
    in_maps = _pack_inputs(**inputs)
    res = run_bass_kernel_spmd(nc, in_maps, core_ids=list(range(B)), trace=trace)
    out = np.stack([
        np.asarray(res.results[i]["out"]).astype(np.float32).reshape(C, H, W)
        for i in range(B)])
    return out, res


def _install_trace_hook():
    import types
    try:
        import antenv.axon_hooks  # noqa
    except ImportError:
        from trn_agent_boot.trn_boot import _ntff_profile_via_ctypes
        hook = _ntff_profile_via_ctypes('/opt/axon/libaxon_pjrt.so')
        mod = types.ModuleType('antenv.axon_hooks')
        mod.get_axon_ntff_profile_hook = lambda: hook
        mod.set_axon_ntff_profile_hook = lambda h: None
        sys.modules['antenv.axon_hooks'] = mod


def kernel(**inputs):
    out, _ = _run(inputs, trace=False)
    return out
